# revision 1
# baseline (speedup 1.0000x reference)
"""Trainium2 Bass kernel for nn_Conv2D_26164940767465.

Per-(channel, filter) VALID 2D cross-correlation with NO channel reduction:
  out[b, ho, c, f, wo] = sum_{i,j} int(x[b, ho+i, wo+j, c]) * int(k[i,j,c,f])

Shapes: x (4,224,224,16) f32 integer-valued [0,256); k (5,5,16,32) f32
integer-valued [-8,8). Output (4,220,16,32,220) f32.

Exactness: x <= 255 and |k| <= 8 are exactly representable in bf16; products
(<= 2040) and 25-tap sums (|.| <= 51000 < 2^24) are exact in the fp32 PSUM
accumulator. So a bf16 tensor-engine matmul reproduces the int32 reference
bit-exactly.

Strategy (8 cores): shard (batch 4) x (output-row halves 2). Per core, for
each group g of 4 channels, one matmul per 2 output rows:
  out[(c_l,f)=128, (row,wo)=440] = WT[g][K=100, 128].T @ XS[g][K=100, 440]
where K = (c_l, i, j) packs the 4 channels' 25 taps and WT is block-diagonal.
XS is a host-built shifted-image (im2col) layout so each matmul needs a single
contiguous rhs slice. PSUM tiles are evacuated to SBUF by the vector/scalar
engines (alternating) and DMA'd out in 10-row chunks.
"""

import os
import sys

if "/opt/trn_rl_repo" not in sys.path:
    sys.path.insert(0, "/opt/trn_rl_repo")

import numpy as np
import ml_dtypes

BF16 = np.dtype(ml_dtypes.bfloat16)

# Problem constants (hardcoded per harness contract).
B, H, W, C = 4, 224, 224, 16
KH, KW, F = 5, 5, 32
HO, WO = H - KH + 1, W - KW + 1          # 220, 220
NCORES = 8
HALF = HO // 2                            # 110 output rows per core
CG = 4                                    # channels per group
NG = C // CG                              # 4 groups
KP = CG * KH * KW                         # 100 contraction rows
MP = CG * F                               # 128 output partitions
ROWS_PER_MM = 2
NMM = HALF // ROWS_PER_MM                 # 55 matmuls per group
NFREE = ROWS_PER_MM * WO                  # 440
ROWS_PER_CHUNK = 10                       # rows staged per output DMA
MM_PER_CHUNK = ROWS_PER_CHUNK // ROWS_PER_MM   # 5
NCHUNK = HALF // ROWS_PER_CHUNK           # 11

_PROGRAM = None


def _build_program():
    import concourse.bacc as bacc
    import concourse.mybir as mybir
    import concourse.tile as tile

    nc = bacc.Bacc("TRN2", target_bir_lowering=False, debug=False,
                   num_devices=NCORES)

    xs_d = nc.dram_tensor("xs", [NG, KP, HALF * WO], mybir.dt.bfloat16,
                          kind="ExternalInput")
    wt_d = nc.dram_tensor("wt", [KP, NG * MP], mybir.dt.bfloat16,
                          kind="ExternalInput")
    out_d = nc.dram_tensor("out", [HALF, C, F, WO], mybir.dt.float32,
                           kind="ExternalOutput")

    xs_ap = xs_d.ap()
    wt_ap = wt_d.ap()
    out_ap = out_d.ap()

    with tile.TileContext(nc) as tc:
        with (
            tc.tile_pool(name="wpool", bufs=1) as wpool,
            tc.tile_pool(name="xpool", bufs=2) as xpool,
            tc.tile_pool(name="spool", bufs=3) as spool,
            tc.tile_pool(name="psum", bufs=8, space="PSUM") as pspool,
        ):
            wt_t = wpool.tile([KP, NG * MP], mybir.dt.bfloat16)
            nc.sync.dma_start(wt_t[:], wt_ap)

            for g in range(NG):
                xs_t = xpool.tile([KP, HALF * WO], mybir.dt.bfloat16)
                nc.sync.dma_start(xs_t[:], xs_ap[g])
                for ch in range(NCHUNK):
                    stage = spool.tile([MP, ROWS_PER_CHUNK * WO],
                                       mybir.dt.float32)
                    for t in range(MM_PER_CHUNK):
                        r = ch * ROWS_PER_CHUNK + t * ROWS_PER_MM
                        ps = pspool.tile([MP, NFREE], mybir.dt.float32)
                        nc.tensor.matmul(
                            ps[:],
                            wt_t[:, g * MP:(g + 1) * MP],
                            xs_t[:, r * WO: r * WO + NFREE],
                            start=True, stop=True,
                        )
                        dst = stage[:, t * NFREE:(t + 1) * NFREE]
                        if t % 2 == 0:
                            nc.vector.tensor_copy(dst, ps[:])
                        else:
                            nc.scalar.copy(dst, ps[:])
                    dram_slab = out_ap[
                        ch * ROWS_PER_CHUNK:(ch + 1) * ROWS_PER_CHUNK,
                        g * CG:(g + 1) * CG, :, :,
                    ].rearrange("r c f w -> (c f) r w")
                    nc.sync.dma_start(
                        dram_slab,
                        stage[:].rearrange("p (r w) -> p r w", w=WO),
                    )

    nc.compile()
    return nc


def _get_program():
    global _PROGRAM
    if _PROGRAM is None:
        _PROGRAM = _build_program()
    return _PROGRAM


def _host_pack(x, k):
    """Build per-core XS tensors and the shared block-diag weights (bf16)."""
    x_bf = np.ascontiguousarray(x.astype(BF16))
    k_bf = k.astype(BF16)

    xs_all = []
    for m in range(NCORES):
        b, half = m // 2, m % 2
        r0 = half * HALF
        # Deinterleave once: [C, 114, 224] channel-major rows.
        xc = np.ascontiguousarray(
            x_bf[b, r0:r0 + HALF + KH - 1].transpose(2, 0, 1))
        xs = np.empty((NG, CG, KH, KW, HALF, WO), dtype=BF16)
        xsv = xs.reshape(C, KH, KW, HALF, WO)
        for i in range(KH):
            for j in range(KW):
                xsv[:, i, j] = xc[:, i:i + HALF, j:j + WO]
        xs_all.append(xs.reshape(NG, KP, HALF * WO))

    wt = np.zeros((KP, NG, MP), dtype=BF16)
    for g in range(NG):
        for cl in range(CG):
            wt[cl * KH * KW:(cl + 1) * KH * KW, g,
               cl * F:(cl + 1) * F] = k_bf[:, :, g * CG + cl, :].reshape(
                   KH * KW, F)
    wt = np.ascontiguousarray(wt.reshape(KP, NG * MP))
    return xs_all, wt


LAST_EXEC_TIME_NS = None


def kernel(**inputs):
    from concourse.bass_utils import run_bass_kernel_spmd

    global LAST_EXEC_TIME_NS
    x = np.asarray(inputs["inputs"])
    k = np.asarray(inputs["kernel"])
    assert x.shape == (B, H, W, C) and k.shape == (KH, KW, C, F)

    nc = _get_program()
    xs_all, wt = _host_pack(x, k)
    in_maps = [{"xs": xs_all[m], "wt": wt} for m in range(NCORES)]

    trace = os.environ.get("CONV_TRACE", "") == "1"
    kwargs = {}
    if trace:
        kwargs["trace"] = True
        tdir = os.environ.get("CONV_TRACE_DIR")
        if tdir:
            kwargs["tmpdir"] = tdir

    res = run_bass_kernel_spmd(nc, in_maps, list(range(NCORES)), **kwargs)
    LAST_EXEC_TIME_NS = res.exec_time_ns

    full = np.empty((B, HO, C, F, WO), dtype=np.float32)
    for m in range(NCORES):
        b, half = m // 2, m % 2
        full[b, half * HALF:(half + 1) * HALF] = res.results[m]["out"]
    return full


# revision 6
# speedup vs baseline: 1.2281x; 1.2281x over previous
"""Trainium2 Bass kernel for nn_Conv2D_26164940767465.

Per-(channel, filter) VALID 2D cross-correlation with NO channel reduction:
  out[b, ho, c, f, wo] = sum_{i,j} int(x[b, ho+i, wo+j, c]) * int(k[i,j,c,f])

Shapes: x (4,224,224,16) f32 integer-valued [0,256); k (5,5,16,32) f32
integer-valued [-8,8). Output (4,220,16,32,220) f32.

Exactness: x <= 255 and |k| <= 8 are exactly representable in bf16; products
(<= 2040) and 25-tap sums (|.| <= 51000 < 2^24) are exact in the fp32 PSUM
accumulator. So a bf16 tensor-engine matmul reproduces the int32 reference
bit-exactly.

Strategy (8 cores): shard (batch 4) x (output-row halves 2). Per core, for
each group g of 4 channels, one matmul per 2 output rows:
  out[(c_l,f)=128, (row,wo)=440] = WT[g][K=100, 128].T @ XS[g][K=100, 440]
where K = (c_l, i, j) packs the 4 channels' 25 taps and WT is block-diagonal.
XS is a host-built shifted-image (im2col) layout so each matmul needs a single
contiguous rhs slice. PSUM tiles are evacuated to SBUF by the vector/scalar
engines (alternating) and DMA'd out in 10-row chunks.
"""

import os
import sys

if "/opt/trn_rl_repo" not in sys.path:
    sys.path.insert(0, "/opt/trn_rl_repo")

import numpy as np
import ml_dtypes

BF16 = np.dtype(ml_dtypes.bfloat16)

# Problem constants (hardcoded per harness contract).
B, H, W, C = 4, 224, 224, 16
KH, KW, F = 5, 5, 32
HO, WO = H - KH + 1, W - KW + 1          # 220, 220
NCORES = 8
HALF = HO // 2                            # 110 output rows per core
CG = 4                                    # channels per group
NG = C // CG                              # 4 groups
KP = CG * KH * KW                         # 100 contraction rows
MP = CG * F                               # 128 output partitions
ROWS_PER_MM = 2
NMM = HALF // ROWS_PER_MM                 # 55 matmuls per group
NFREE = ROWS_PER_MM * WO                  # 440
ROWS_PER_CHUNK = 22                       # rows staged per output DMA
MM_PER_CHUNK = ROWS_PER_CHUNK // ROWS_PER_MM   # 11
NCHUNK = HALF // ROWS_PER_CHUNK           # 5

_PROGRAM = None


def _build_program():
    import concourse.bacc as bacc
    import concourse.mybir as mybir
    import concourse.tile as tile

    nc = bacc.Bacc("TRN2", target_bir_lowering=False, debug=False,
                   num_devices=NCORES)

    xs_d = nc.dram_tensor("xs", [NG, KP, HALF * WO], mybir.dt.bfloat16,
                          kind="ExternalInput")
    wt_d = nc.dram_tensor("wt", [KP, NG * MP], mybir.dt.bfloat16,
                          kind="ExternalInput")
    # [C, F, rows, wo] layout: each output partition (c_l, f) owns a fully
    # contiguous DRAM run per chunk (ROWS_PER_CHUNK*WO*4 = 19.4 KB
    # descriptors instead of 880 B). Host transposes back on assembly.
    out_d = nc.dram_tensor("out", [C, F, HALF, WO], mybir.dt.float32,
                           kind="ExternalOutput")

    xs_ap = xs_d.ap()
    wt_ap = wt_d.ap()
    out_ap = out_d.ap()

    with tile.TileContext(nc) as tc:
        with (
            tc.tile_pool(name="wpool", bufs=1) as wpool,
            tc.tile_pool(name="xpool", bufs=2) as xpool,
            tc.tile_pool(name="spool", bufs=3) as spool,
            tc.tile_pool(name="psum", bufs=8, space="PSUM") as pspool,
        ):
            # Inputs go through the Scalar HWDGE queue so the big XS loads
            # never sit in front of output chunks on the Sync queue's FIFO.
            wt_t = wpool.tile([KP, NG * MP], mybir.dt.bfloat16)
            nc.scalar.dma_start(wt_t[:], wt_ap)

            for g in range(NG):
                xs_t = xpool.tile([KP, HALF * WO], mybir.dt.bfloat16)
                nc.scalar.dma_start(xs_t[:], xs_ap[g])
                for ch in range(NCHUNK):
                    stage = spool.tile([MP, ROWS_PER_CHUNK * WO],
                                       mybir.dt.float32)
                    for t in range(MM_PER_CHUNK):
                        r = ch * ROWS_PER_CHUNK + t * ROWS_PER_MM
                        ps = pspool.tile([MP, NFREE], mybir.dt.float32)
                        nc.tensor.matmul(
                            ps[:],
                            wt_t[:, g * MP:(g + 1) * MP],
                            xs_t[:, r * WO: r * WO + NFREE],
                            start=True, stop=True,
                        )
                        dst = stage[:, t * NFREE:(t + 1) * NFREE]
                        if t % 2 == 0:
                            nc.vector.tensor_copy(dst, ps[:])
                        else:
                            nc.scalar.copy(dst, ps[:])
                    dram_slab = out_ap[
                        g * CG:(g + 1) * CG, :,
                        ch * ROWS_PER_CHUNK:(ch + 1) * ROWS_PER_CHUNK, :,
                    ].rearrange("c f r w -> (c f) r w")
                    nc.sync.dma_start(
                        dram_slab,
                        stage[:].rearrange("p (r w) -> p r w", w=WO),
                    )

    nc.compile()
    return nc


def _get_program():
    global _PROGRAM
    if _PROGRAM is None:
        _PROGRAM = _build_program()
    return _PROGRAM


def _host_pack(x, k):
    """Build per-core XS tensors and the shared block-diag weights (bf16)."""
    x_bf = np.ascontiguousarray(x.astype(BF16))
    k_bf = k.astype(BF16)

    xs_all = []
    for m in range(NCORES):
        b, half = m // 2, m % 2
        r0 = half * HALF
        # Deinterleave once: [C, 114, 224] channel-major rows.
        xc = np.ascontiguousarray(
            x_bf[b, r0:r0 + HALF + KH - 1].transpose(2, 0, 1))
        xs = np.empty((NG, CG, KH, KW, HALF, WO), dtype=BF16)
        xsv = xs.reshape(C, KH, KW, HALF, WO)
        for i in range(KH):
            for j in range(KW):
                xsv[:, i, j] = xc[:, i:i + HALF, j:j + WO]
        xs_all.append(xs.reshape(NG, KP, HALF * WO))

    wt = np.zeros((KP, NG, MP), dtype=BF16)
    for g in range(NG):
        for cl in range(CG):
            wt[cl * KH * KW:(cl + 1) * KH * KW, g,
               cl * F:(cl + 1) * F] = k_bf[:, :, g * CG + cl, :].reshape(
                   KH * KW, F)
    wt = np.ascontiguousarray(wt.reshape(KP, NG * MP))
    return xs_all, wt


LAST_EXEC_TIME_NS = None


def kernel(**inputs):
    from concourse.bass_utils import run_bass_kernel_spmd

    global LAST_EXEC_TIME_NS
    x = np.asarray(inputs["inputs"])
    k = np.asarray(inputs["kernel"])
    assert x.shape == (B, H, W, C) and k.shape == (KH, KW, C, F)

    nc = _get_program()
    xs_all, wt = _host_pack(x, k)
    in_maps = [{"xs": xs_all[m], "wt": wt} for m in range(NCORES)]

    trace = os.environ.get("CONV_TRACE", "") == "1"
    kwargs = {}
    if trace:
        kwargs["trace"] = True
        tdir = os.environ.get("CONV_TRACE_DIR")
        if tdir:
            kwargs["tmpdir"] = tdir

    res = run_bass_kernel_spmd(nc, in_maps, list(range(NCORES)), **kwargs)
    LAST_EXEC_TIME_NS = res.exec_time_ns

    full = np.empty((B, HO, C, F, WO), dtype=np.float32)
    for m in range(NCORES):
        b, half = m // 2, m % 2
        # device layout [C, F, rows, WO] -> reference layout [rows, C, F, WO]
        full[b, half * HALF:(half + 1) * HALF] = \
            res.results[m]["out"].transpose(2, 0, 1, 3)
    return full


# revision 8
# speedup vs baseline: 1.6896x; 1.3758x over previous
"""Trainium2 Bass kernel for nn_Conv2D_26164940767465.

Per-(channel, filter) VALID 2D cross-correlation with NO channel reduction:
  out[b, ho, c, f, wo] = sum_{i,j} int(x[b, ho+i, wo+j, c]) * int(k[i,j,c,f])

Shapes: x (4,224,224,16) f32 integer-valued [0,256); k (5,5,16,32) f32
integer-valued [-8,8). Output (4,220,16,32,220) f32.

Exactness: x <= 255 and |k| <= 8 are exactly representable in bf16; products
(<= 2040) and 25-tap sums (|.| <= 51000 < 2^24) are exact in the fp32 PSUM
accumulator. So a bf16 tensor-engine matmul reproduces the int32 reference
bit-exactly.

Strategy (8 cores): shard (batch 4) x (output-row halves 2). Per core the
input lives in SBUF once, as a j-shifted channel-major buffer XSJ on a padded
128-partition layout: partition 32*g + c_l*5 + j holds row-major image rows of
channel c = 4*g + c_l shifted left by j. For every pair of output rows, the
four channel groups g run CONCURRENTLY as 4 row-tiled matmuls (K=20 each,
tile_position=(32g,0)) into 4 PSUM banks, accumulating the 5 kernel rows i
via rhs free-offset (r+i)*WO — no im2col materialization at all. PSUM tiles
are evacuated by vector/scalar engines and written out in 10-row chunks whose
per-partition DRAM runs are fully contiguous (out layout [C, F, rows, wo]).
"""

import os
import sys

if "/opt/trn_rl_repo" not in sys.path:
    sys.path.insert(0, "/opt/trn_rl_repo")

import numpy as np
import ml_dtypes

BF16 = np.dtype(ml_dtypes.bfloat16)

# Problem constants (hardcoded per harness contract).
B, H, W, C = 4, 224, 224, 16
KH, KW, F = 5, 5, 32
HO, WO = H - KH + 1, W - KW + 1          # 220, 220
NCORES = 8
HALF = HO // 2                            # 110 output rows per core
HIN = HALF + KH - 1                       # 114 input rows per core
CG = 4                                    # channels per group
NG = C // CG                              # 4 groups
KJ = CG * KW                              # 20 contraction rows per group
MP = CG * F                               # 128 output partitions
ROWS_PER_MM = 2
NFREE = ROWS_PER_MM * WO                  # 440
ROWS_PER_CHUNK = 10                       # rows staged per output DMA
MM_PER_CHUNK = ROWS_PER_CHUNK // ROWS_PER_MM   # 5 row-pairs
NCHUNK = HALF // ROWS_PER_CHUNK           # 11

_PROGRAM = None


def _build_program():
    import concourse.bacc as bacc
    import concourse.mybir as mybir
    import concourse.tile as tile

    nc = bacc.Bacc("TRN2", target_bir_lowering=False, debug=False,
                   num_devices=NCORES)

    xsj_d = nc.dram_tensor("xsj", [128, HIN * WO], mybir.dt.bfloat16,
                           kind="ExternalInput")
    wt_d = nc.dram_tensor("wt", [128, KH * MP], mybir.dt.bfloat16,
                          kind="ExternalInput")
    # [C, F, rows, wo] layout: each output partition (c_l, f) owns a fully
    # contiguous DRAM run per chunk. Host transposes back on assembly.
    out_d = nc.dram_tensor("out", [C, F, HALF, WO], mybir.dt.float32,
                           kind="ExternalOutput")

    xsj_ap = xsj_d.ap()
    wt_ap = wt_d.ap()
    out_ap = out_d.ap()

    with tile.TileContext(nc) as tc:
        with (
            tc.tile_pool(name="wpool", bufs=1) as wpool,
            tc.tile_pool(name="xpool", bufs=1) as xpool,
            tc.tile_pool(name="spool", bufs=2) as spool,
            tc.tile_pool(name="psum", bufs=2, space="PSUM") as pspool,
        ):
            # Inputs on the Scalar HWDGE queue; outputs on Sync, so the big
            # input load never sits in front of output chunks in a FIFO.
            wt_t = wpool.tile([128, KH * MP], mybir.dt.bfloat16)
            nc.scalar.dma_start(wt_t[:], wt_ap)
            xsj_t = xpool.tile([128, HIN * WO], mybir.dt.bfloat16)
            nc.scalar.dma_start(xsj_t[:], xsj_ap)

            for ch in range(NCHUNK):
                stages = [
                    spool.tile([MP, ROWS_PER_CHUNK * WO], mybir.dt.float32,
                               tag=f"stage{g}", name=f"stage{g}")
                    for g in range(NG)
                ]
                for t in range(MM_PER_CHUNK):
                    r = ch * ROWS_PER_CHUNK + t * ROWS_PER_MM
                    pss = [
                        pspool.tile([MP, NFREE], mybir.dt.float32,
                                    tag=f"ps{g}", name=f"ps{g}")
                        for g in range(NG)
                    ]
                    for i in range(KH):
                        off = (r + i) * WO
                        for g in range(NG):
                            p0 = 32 * g
                            nc.tensor.matmul(
                                pss[g][:],
                                wt_t[p0:p0 + KJ, i * MP:(i + 1) * MP],
                                xsj_t[p0:p0 + KJ, off:off + NFREE],
                                start=(i == 0), stop=(i == KH - 1),
                                tile_position=(p0, 0),
                            )
                    for g in range(NG):
                        dst = stages[g][:, t * NFREE:(t + 1) * NFREE]
                        if (t * NG + g) % 2 == 0:
                            nc.vector.tensor_copy(dst, pss[g][:])
                        else:
                            nc.scalar.copy(dst, pss[g][:])
                for g in range(NG):
                    dram_slab = out_ap[
                        g * CG:(g + 1) * CG, :,
                        ch * ROWS_PER_CHUNK:(ch + 1) * ROWS_PER_CHUNK, :,
                    ].rearrange("c f r w -> (c f) r w")
                    nc.sync.dma_start(
                        dram_slab,
                        stages[g][:].rearrange("p (r w) -> p r w", w=WO),
                    )

    nc.compile()
    return nc


def _get_program():
    global _PROGRAM
    if _PROGRAM is None:
        _PROGRAM = _build_program()
    return _PROGRAM


def _host_pack(x, k):
    """Build per-core XSJ tensors and the shared per-tap weights (bf16)."""
    x_bf = np.ascontiguousarray(x.astype(BF16))
    k_bf = k.astype(BF16)

    xsj_all = []
    for m in range(NCORES):
        b, half = m // 2, m % 2
        r0 = half * HALF
        # Deinterleave once: [C, 114, 224] channel-major rows.
        xc = np.ascontiguousarray(x_bf[b, r0:r0 + HIN].transpose(2, 0, 1))
        xp = np.zeros((128, HIN, WO), dtype=BF16)
        for c in range(C):
            g, cl = c // CG, c % CG
            base = 32 * g + cl * KW
            for j in range(KW):
                xp[base + j] = xc[c, :, j:j + WO]
        xsj_all.append(xp.reshape(128, HIN * WO))

    wt = np.zeros((128, KH, MP), dtype=BF16)
    for c in range(C):
        g, cl = c // CG, c % CG
        base = 32 * g + cl * KW
        for j in range(KW):
            for i in range(KH):
                wt[base + j, i, cl * F:(cl + 1) * F] = k_bf[i, j, c, :]
    wt = np.ascontiguousarray(wt.reshape(128, KH * MP))
    return xsj_all, wt


LAST_EXEC_TIME_NS = None


def kernel(**inputs):
    from concourse.bass_utils import run_bass_kernel_spmd

    global LAST_EXEC_TIME_NS
    x = np.asarray(inputs["inputs"])
    k = np.asarray(inputs["kernel"])
    assert x.shape == (B, H, W, C) and k.shape == (KH, KW, C, F)

    nc = _get_program()
    xsj_all, wt = _host_pack(x, k)
    in_maps = [{"xsj": xsj_all[m], "wt": wt} for m in range(NCORES)]

    trace = os.environ.get("CONV_TRACE", "") == "1"
    kwargs = {}
    if trace:
        kwargs["trace"] = True
        tdir = os.environ.get("CONV_TRACE_DIR")
        if tdir:
            kwargs["tmpdir"] = tdir

    res = run_bass_kernel_spmd(nc, in_maps, list(range(NCORES)), **kwargs)
    LAST_EXEC_TIME_NS = res.exec_time_ns

    full = np.empty((B, HO, C, F, WO), dtype=np.float32)
    for m in range(NCORES):
        b, half = m // 2, m % 2
        # device layout [C, F, rows, WO] -> reference layout [rows, C, F, WO]
        full[b, half * HALF:(half + 1) * HALF] = \
            res.results[m]["out"].transpose(2, 0, 1, 3)
    return full


# revision 10
# speedup vs baseline: 1.7436x; 1.0320x over previous
"""Trainium2 Bass kernel for nn_Conv2D_26164940767465.

Per-(channel, filter) VALID 2D cross-correlation with NO channel reduction:
  out[b, ho, c, f, wo] = sum_{i,j} int(x[b, ho+i, wo+j, c]) * int(k[i,j,c,f])

Shapes: x (4,224,224,16) f32 integer-valued [0,256); k (5,5,16,32) f32
integer-valued [-8,8). Output (4,220,16,32,220) f32.

Exactness: x <= 255 and |k| <= 8 are exactly representable in bf16; products
(<= 2040) and 25-tap sums (|.| <= 51000 < 2^24) are exact in the fp32 PSUM
accumulator. So a bf16 tensor-engine matmul reproduces the int32 reference
bit-exactly.

Strategy (8 cores): shard (batch 4) x (output-row halves 2). Per core the
input lives in SBUF once, as a j-shifted channel-major buffer XSJ on a padded
128-partition layout: partition 32*g + c_l*5 + j holds row-major image rows of
channel c = 4*g + c_l shifted left by j. For every pair of output rows, the
four channel groups g run CONCURRENTLY as 4 row-tiled matmuls (K=20 each,
tile_position=(32g,0)) into 4 PSUM banks, accumulating the 5 kernel rows i
via rhs free-offset (r+i)*WO — no im2col materialization at all. PSUM tiles
are evacuated by vector/scalar engines and written out in 10-row chunks whose
per-partition DRAM runs are fully contiguous (out layout [C, F, rows, wo]).
"""

import os
import sys

if "/opt/trn_rl_repo" not in sys.path:
    sys.path.insert(0, "/opt/trn_rl_repo")

import numpy as np
import ml_dtypes

BF16 = np.dtype(ml_dtypes.bfloat16)

# Problem constants (hardcoded per harness contract).
B, H, W, C = 4, 224, 224, 16
KH, KW, F = 5, 5, 32
HO, WO = H - KH + 1, W - KW + 1          # 220, 220
NCORES = 8
HALF = HO // 2                            # 110 output rows per core
HIN = HALF + KH - 1                       # 114 input rows per core
CG = 4                                    # channels per group
NG = C // CG                              # 4 groups
KJ = CG * KW                              # 20 contraction rows per group
MP = CG * F                               # 128 output partitions
ROWS_PER_MM = 2
NFREE = ROWS_PER_MM * WO                  # 440
ROWS_PER_CHUNK = 10                       # rows staged per output DMA
MM_PER_CHUNK = ROWS_PER_CHUNK // ROWS_PER_MM   # 5 row-pairs
NCHUNK = HALF // ROWS_PER_CHUNK           # 11

_PROGRAM = None


def _build_program():
    import concourse.bacc as bacc
    import concourse.mybir as mybir
    import concourse.tile as tile

    nc = bacc.Bacc("TRN2", target_bir_lowering=False, debug=False,
                   num_devices=NCORES)

    xsj_d = nc.dram_tensor("xsj", [128, HIN * WO], mybir.dt.bfloat16,
                           kind="ExternalInput")
    wt_d = nc.dram_tensor("wt", [128, KH * MP], mybir.dt.bfloat16,
                          kind="ExternalInput")
    # [C, F, rows, wo] layout: each output partition (c_l, f) owns a fully
    # contiguous DRAM run per chunk. Host transposes back on assembly.
    out_d = nc.dram_tensor("out", [C, F, HALF, WO], mybir.dt.float32,
                           kind="ExternalOutput")

    xsj_ap = xsj_d.ap()
    wt_ap = wt_d.ap()
    out_ap = out_d.ap()

    with tile.TileContext(nc) as tc:
        with (
            tc.tile_pool(name="wpool", bufs=1) as wpool,
            tc.tile_pool(name="xpool", bufs=1) as xpool,
            tc.tile_pool(name="spool", bufs=3) as spool,
            tc.tile_pool(name="psum", bufs=2, space="PSUM") as pspool,
        ):
            # Inputs on the Scalar HWDGE queue; outputs on Sync, so the big
            # input load never sits in front of output chunks in a FIFO.
            wt_t = wpool.tile([128, KH * MP], mybir.dt.bfloat16)
            nc.scalar.dma_start(wt_t[:], wt_ap)
            xsj_t = xpool.tile([128, HIN * WO], mybir.dt.bfloat16)
            # Load in row-range pieces so the first chunks' matmuls start
            # after ~1/4 of the input has landed instead of all of it.
            row_splits = [0, 29, 58, 87, HIN]
            for a, b in zip(row_splits, row_splits[1:]):
                nc.scalar.dma_start(xsj_t[:, a * WO:b * WO],
                                    xsj_ap[:, a * WO:b * WO])

            for ch in range(NCHUNK):
                stages = [
                    spool.tile([MP, ROWS_PER_CHUNK * WO], mybir.dt.float32,
                               tag=f"stage{g}", name=f"stage{g}")
                    for g in range(NG)
                ]
                for t in range(MM_PER_CHUNK):
                    r = ch * ROWS_PER_CHUNK + t * ROWS_PER_MM
                    pss = [
                        pspool.tile([MP, NFREE], mybir.dt.float32,
                                    tag=f"ps{g}", name=f"ps{g}")
                        for g in range(NG)
                    ]
                    for i in range(KH):
                        off = (r + i) * WO
                        for g in range(NG):
                            p0 = 32 * g
                            nc.tensor.matmul(
                                pss[g][:],
                                wt_t[p0:p0 + KJ, i * MP:(i + 1) * MP],
                                xsj_t[p0:p0 + KJ, off:off + NFREE],
                                start=(i == 0), stop=(i == KH - 1),
                                tile_position=(p0, 0),
                            )
                    for g in range(NG):
                        dst = stages[g][:, t * NFREE:(t + 1) * NFREE]
                        if (t * NG + g) % 2 == 0:
                            nc.vector.tensor_copy(dst, pss[g][:])
                        else:
                            nc.scalar.copy(dst, pss[g][:])
                for g in range(NG):
                    dram_slab = out_ap[
                        g * CG:(g + 1) * CG, :,
                        ch * ROWS_PER_CHUNK:(ch + 1) * ROWS_PER_CHUNK, :,
                    ].rearrange("c f r w -> (c f) r w")
                    nc.sync.dma_start(
                        dram_slab,
                        stages[g][:].rearrange("p (r w) -> p r w", w=WO),
                    )

    nc.compile()
    return nc


def _get_program():
    global _PROGRAM
    if _PROGRAM is None:
        _PROGRAM = _build_program()
    return _PROGRAM


def _host_pack(x, k):
    """Build per-core XSJ tensors and the shared per-tap weights (bf16)."""
    x_bf = np.ascontiguousarray(x.astype(BF16))
    k_bf = k.astype(BF16)

    xsj_all = []
    for m in range(NCORES):
        b, half = m // 2, m % 2
        r0 = half * HALF
        # Deinterleave once: [C, 114, 224] channel-major rows.
        xc = np.ascontiguousarray(x_bf[b, r0:r0 + HIN].transpose(2, 0, 1))
        xp = np.zeros((128, HIN, WO), dtype=BF16)
        for c in range(C):
            g, cl = c // CG, c % CG
            base = 32 * g + cl * KW
            for j in range(KW):
                xp[base + j] = xc[c, :, j:j + WO]
        xsj_all.append(xp.reshape(128, HIN * WO))

    wt = np.zeros((128, KH, MP), dtype=BF16)
    for c in range(C):
        g, cl = c // CG, c % CG
        base = 32 * g + cl * KW
        for j in range(KW):
            for i in range(KH):
                wt[base + j, i, cl * F:(cl + 1) * F] = k_bf[i, j, c, :]
    wt = np.ascontiguousarray(wt.reshape(128, KH * MP))
    return xsj_all, wt


LAST_EXEC_TIME_NS = None


def kernel(**inputs):
    from concourse.bass_utils import run_bass_kernel_spmd

    global LAST_EXEC_TIME_NS
    x = np.asarray(inputs["inputs"])
    k = np.asarray(inputs["kernel"])
    assert x.shape == (B, H, W, C) and k.shape == (KH, KW, C, F)

    nc = _get_program()
    xsj_all, wt = _host_pack(x, k)
    in_maps = [{"xsj": xsj_all[m], "wt": wt} for m in range(NCORES)]

    trace = os.environ.get("CONV_TRACE", "") == "1"
    kwargs = {}
    if trace:
        kwargs["trace"] = True
        tdir = os.environ.get("CONV_TRACE_DIR")
        if tdir:
            kwargs["tmpdir"] = tdir

    res = run_bass_kernel_spmd(nc, in_maps, list(range(NCORES)), **kwargs)
    LAST_EXEC_TIME_NS = res.exec_time_ns

    full = np.empty((B, HO, C, F, WO), dtype=np.float32)
    for m in range(NCORES):
        b, half = m // 2, m % 2
        # device layout [C, F, rows, WO] -> reference layout [rows, C, F, WO]
        full[b, half * HALF:(half + 1) * HALF] = \
            res.results[m]["out"].transpose(2, 0, 1, 3)
    return full
